# revision 4
# baseline (speedup 1.0000x reference)
"""Trainium2 Bass kernel for nn_Attention_72791105732908 (sparse_attention).

Reference computation (L=2048, B=64, H=1024, HC=1024):
    outs   = prev_layer_outputs.transpose(1, 0, 2)              # [B, L, H]
    energy = tanh(concat([hidden_bcast, outs], -1) @ W_e.T + b_e)  # [B, L, HC]
    attn   = energy @ W_v                                        # [B, L]
    attn   = where(mask == 0, -1e10, attn); softmax over L
    out    = einsum('bl,blh->bh', attn, outs)[None]              # [1, B, H]

Strategy:
  - Data-parallel over batch: core i handles batches 8i..8i+7. No collectives.
  - Split the concat matmul: q[b] = hidden[b] @ W_h.T + b_e is computed once
    per batch (tiny), the big matmul is outs @ W_o.T (halves the FLOPs).
  - bf16 on the PE for the big matmuls (fp32 PSUM accumulation).
  - outs arrives [L, b, H]; the energy matmul contracts over H, so outs is
    transposed to [H, L] tiles with the DMA xbar (2-byte dtype, DRAM->SBUF,
    mapping T[p, j, l] = outs[l, 128j + p]). Host pre-permutes W to match.
  - Masked softmax without max-subtraction (scores are bounded: |s| <= 32):
    w = exp(s) * mask; normalization folded into the output scale.
  - Score row [1, 2048] is moved onto partitions via K=1 matmuls with a
    ones [1, 1] rhs (exact), giving lhsT columns for the weighted sum.
"""
import numpy as np
import ml_dtypes

import concourse.bacc as bacc
import concourse.mybir as mybir
import concourse.tile as tile
from concourse.bass_utils import run_bass_kernel_spmd

dt = mybir.dt
AF = mybir.ActivationFunctionType

L, B, H, HC = 2048, 64, 1024, 1024
NCORES = 8
BPC = B // NCORES        # batches per core
P = 128
LC = L // P              # 16 l-chunks
JH = H // P              # 8 h-chunks
MC = HC // P             # 8 c-chunks
L4 = L // 512            # 4 chunks of 512 along L
BF = ml_dtypes.bfloat16

_CACHE = {}


def _build():
    nc = bacc.Bacc()
    prev = nc.dram_tensor("prev", [L, BPC, H], dt.bfloat16, kind="ExternalInput")
    WoT = nc.dram_tensor("WoT", [P, JH, HC], dt.bfloat16, kind="ExternalInput")
    WhT = nc.dram_tensor("WhT", [P, JH, HC], dt.bfloat16, kind="ExternalInput")
    hT = nc.dram_tensor("hT", [P, JH, BPC], dt.bfloat16, kind="ExternalInput")
    WvT = nc.dram_tensor("WvT", [P, MC], dt.bfloat16, kind="ExternalInput")
    beT = nc.dram_tensor("beT", [P, MC], dt.float32, kind="ExternalInput")
    mskT = nc.dram_tensor("mskT", [P, BPC, LC], dt.float32, kind="ExternalInput")
    out = nc.dram_tensor("out", [BPC, H], dt.float32, kind="ExternalOutput")

    with tile.TileContext(nc) as tc:
        with (
            tc.tile_pool(name="const", bufs=1) as const,
            tc.tile_pool(name="data", bufs=2) as data,
            tc.tile_pool(name="et", bufs=3) as etp,
            tc.tile_pool(name="small", bufs=2) as small,
            tc.tile_pool(name="pse", bufs=2, space="PSUM") as pse_p,
            tc.tile_pool(name="pss", bufs=2, space="PSUM") as pss_p,
            tc.tile_pool(name="psw", bufs=2, space="PSUM") as psw_p,
            tc.tile_pool(name="pwo", bufs=2, space="PSUM") as pwo_p,
        ):
            # ---- constants
            wo = const.tile([P, JH, HC], dt.bfloat16)
            nc.sync.dma_start(out=wo[:], in_=WoT[:])
            wh = const.tile([P, JH, HC], dt.bfloat16)
            nc.sync.dma_start(out=wh[:], in_=WhT[:])
            ht = const.tile([P, JH, BPC], dt.bfloat16)
            nc.sync.dma_start(out=ht[:], in_=hT[:])
            wv = const.tile([P, MC], dt.bfloat16)
            nc.sync.dma_start(out=wv[:], in_=WvT[:])
            be = const.tile([P, MC], dt.float32)
            nc.sync.dma_start(out=be[:], in_=beT[:])
            mk = const.tile([P, BPC, LC], dt.float32)
            nc.sync.dma_start(out=mk[:], in_=mskT[:])
            ones1 = const.tile([1, 1], dt.float32)
            nc.vector.memset(ones1[:], 1.0)
            onesp = const.tile([P, 1], dt.bfloat16)
            nc.vector.memset(onesp[:], 1.0)

            # ---- q[b, c] = hidden[b] @ W_h.T + b_e, laid out [c-part, m, b]
            qb = const.tile([P, MC, BPC], dt.float32)
            for m in range(MC):
                psq = pse_p.tile([P, 512], dt.float32, tag="pse")
                for u in range(JH):
                    nc.tensor.matmul(
                        psq[:, :BPC],
                        wh[:, u, m * P:(m + 1) * P],
                        ht[:, u, :],
                        start=(u == 0), stop=(u == JH - 1),
                    )
                nc.vector.tensor_scalar_add(qb[:, m, :], psq[:, :BPC], be[:, m:m + 1])

            # ---- per-batch pipeline, software-pipelined on the PE queue:
            #  * each scores-MM is deferred until after the NEXT energy block,
            #    so its tanh dependency is off the PE critical path;
            #  * each batch's tail (exp -> K=1 transpose -> softmax -> weighted
            #    sum) is deferred into the next batch's first l4 block.
            pend_s = [None]   # deferred scores-MM thunk
            pend_tail = [None]  # deferred batch-tail thunk

            def emit_pend_s():
                if pend_s[0] is not None:
                    pend_s[0]()
                    pend_s[0] = None

            def make_tail(b, es, nat):
                def tail():
                    # move scores onto partitions: wT[p, c] = es[128c + p]
                    psw = psw_p.tile([P, LC], dt.float32, tag="psw")
                    for c in range(LC):
                        nc.tensor.matmul(
                            psw[:, c:c + 1],
                            es[0:1, c * P:(c + 1) * P],
                            ones1[:],
                            start=True, stop=True,
                        )
                    wtf = small.tile([P, LC], dt.float32, tag="wtf")
                    nc.vector.tensor_mul(wtf[:], psw[:], mk[:, b, :])
                    wt = small.tile([P, LC], dt.bfloat16, tag="wt")
                    nc.vector.tensor_copy(wt[:], wtf[:])

                    # sum of weights (of the bf16-rounded values actually used)
                    pssum = psw_p.tile([1, LC], dt.float32, tag="psw")
                    nc.tensor.matmul(pssum[:], onesp[:], wt[:], start=True, stop=True)
                    ssum = small.tile([1, 1], dt.float32, tag="ssum")
                    nc.vector.reduce_sum(ssum[:], pssum[:], axis=mybir.AxisListType.X)
                    rsum = small.tile([1, 1], dt.float32, tag="rsum")
                    nc.vector.reciprocal(rsum[:], ssum[:])

                    # weighted sum over L, normalized by scale on the way out
                    ob = small.tile([1, H], dt.float32, tag="ob")
                    for half in range(2):
                        hsl = slice(half * 512, (half + 1) * 512)
                        pwo = pwo_p.tile([1, 512], dt.float32, tag="pwo")
                        for c in range(LC):
                            nc.tensor.matmul(
                                pwo[:],
                                wt[:, c:c + 1],
                                nat[:, c, hsl],
                                start=(c == 0), stop=(c == LC - 1),
                            )
                        nc.scalar.activation(ob[0:1, hsl], pwo[:], AF.Copy, scale=rsum[0:1, :])
                    nc.sync.dma_start(out=out[b:b + 1, :], in_=ob[:])
                return tail

            for b in range(BPC):
                # transposed activations: T[p, j, l] = outs[l, 128j + p]
                tb = data.tile([P, JH, L], dt.bfloat16, tag="tb")
                for c in range(LC):
                    nc.sync.dma_start(
                        out=tb[:, :, c * P:(c + 1) * P],
                        in_=prev[c * P:(c + 1) * P, b, :],
                        transpose=True,
                    )
                # natural activations for the weighted sum: nat[p, c, h] = outs[128c + p, h]
                nat = data.tile([P, LC, H], dt.bfloat16, tag="nat")
                nc.sync.dma_start(
                    out=nat[:],
                    in_=prev[:, b, :].rearrange("(c p) h -> p c h", p=P),
                )

                es = small.tile([1, L], dt.float32, tag="es")
                for l4 in range(L4):
                    lsl = slice(l4 * 512, (l4 + 1) * 512)
                    pss = pss_p.tile([1, 512], dt.float32, tag="pss")
                    for m in range(MC):
                        pse = pse_p.tile([P, 512], dt.float32, tag="pse")
                        for j in range(JH):
                            nc.tensor.matmul(
                                pse[:],
                                wo[:, j, m * P:(m + 1) * P],
                                tb[:, j, lsl],
                                start=(j == 0), stop=(j == JH - 1),
                            )
                        emit_pend_s()
                        et = etp.tile([P, 512], dt.bfloat16, tag="et")
                        nc.scalar.activation(et[:], pse[:], AF.Tanh, bias=qb[:, m, b:b + 1])

                        def make_s(et=et, pss=pss, m=m, es=es, lsl=lsl):
                            def s():
                                nc.tensor.matmul(
                                    pss[:],
                                    wv[:, m:m + 1],
                                    et[:],
                                    start=(m == 0), stop=(m == MC - 1),
                                )
                                if m == MC - 1:
                                    nc.scalar.activation(es[0:1, lsl], pss[:], AF.Exp)
                            return s
                        pend_s[0] = make_s()
                    if l4 == 0 and pend_tail[0] is not None:
                        pend_tail[0]()
                        pend_tail[0] = None
                pend_tail[0] = make_tail(b, es, nat)

            emit_pend_s()
            pend_tail[0]()

    nc.finalize()
    return nc


def _in_maps(prev_layer_outputs, hidden, mask, W_e, b_e, W_v):
    # host-side layout prep (cheap, O(MB) except the bf16 cast of prev)
    WoT = np.ascontiguousarray(
        W_e[:, H:].T.reshape(JH, P, HC).transpose(1, 0, 2)).astype(BF)
    WhT = np.ascontiguousarray(
        W_e[:, :H].T.reshape(JH, P, HC).transpose(1, 0, 2)).astype(BF)
    hT_full = np.ascontiguousarray(
        hidden.T.reshape(JH, P, B).transpose(1, 0, 2)).astype(BF)
    WvT = np.ascontiguousarray(W_v.reshape(MC, P).T).astype(BF)
    beT = np.ascontiguousarray(b_e.reshape(MC, P).T).astype(np.float32)

    in_maps = []
    for i in range(NCORES):
        bs = slice(i * BPC, (i + 1) * BPC)
        prev_i = np.ascontiguousarray(prev_layer_outputs[:, bs, :]).astype(BF)
        mskT_i = np.ascontiguousarray(
            mask[bs, :].reshape(BPC, LC, P).transpose(2, 0, 1)).astype(np.float32)
        hT_i = np.ascontiguousarray(hT_full[:, :, bs])
        in_maps.append({
            "prev": prev_i, "WoT": WoT, "WhT": WhT, "hT": hT_i,
            "WvT": WvT, "beT": beT, "mskT": mskT_i,
        })
    return in_maps


def kernel(prev_layer_outputs, hidden, mask, W_e, b_e, W_v):
    if "nc" not in _CACHE:
        _CACHE["nc"] = _build()
    nc = _CACHE["nc"]
    in_maps = _in_maps(prev_layer_outputs, hidden, mask, W_e, b_e, W_v)
    res = run_bass_kernel_spmd(nc, in_maps, list(range(NCORES)))
    out = np.concatenate([np.asarray(r["out"]) for r in res.results], axis=0)
    return out[None, :, :].astype(np.float32)


def run_traced(inputs):
    """Profiled run (test harness only)."""
    if "nc" not in _CACHE:
        _CACHE["nc"] = _build()
    nc = _CACHE["nc"]
    in_maps = _in_maps(**inputs)
    return run_bass_kernel_spmd(nc, in_maps, list(range(NCORES)), trace=True)


# revision 18
# speedup vs baseline: 1.0911x; 1.0911x over previous
"""Trainium2 Bass kernel for nn_Attention_72791105732908 (sparse_attention).

Reference computation (L=2048, B=64, H=1024, HC=1024):
    outs   = prev_layer_outputs.transpose(1, 0, 2)              # [B, L, H]
    energy = tanh(concat([hidden_bcast, outs], -1) @ W_e.T + b_e)  # [B, L, HC]
    attn   = energy @ W_v                                        # [B, L]
    attn   = where(mask == 0, -1e10, attn); softmax over L
    out    = einsum('bl,blh->bh', attn, outs)[None]              # [1, B, H]

Strategy:
  - Data-parallel over batch: core i handles batches 8i..8i+7. No collectives.
  - Split the concat matmul: q[b] = hidden[b] @ W_h.T + b_e is computed once
    per batch (tiny); the big matmul is outs @ W_o.T (halves the FLOPs).
  - bf16 on the PE with fp32 PSUM accumulation.
  - outs arrives [L, b, H]; the energy matmul contracts over H, so outs is
    transposed to [H, L] tiles with the DMA xbar (2-byte dtype, DRAM->SBUF,
    mapping T[p, j, l] = outs[l, 128j + p]). Host pre-permutes W to match.
  - Masked softmax without max-subtraction (scores are bounded: |s| <= 32):
    w = exp(s) * mask; the normalization is applied to the reduced output.
  - The weighted sum over L runs on the (otherwise idle) vector engine as
    tensor_tensor_reduce over the transposed tiles, with the masked weights
    broadcast to all partitions by a K=1 ones matmul. This keeps the tensor
    engine free for the energy matmul, which is the roofline.
  - All cross-engine consumers of PE results are deferred on the PE queue
    (scores-MMs by one energy block; softmax/weighted-sum chunks by two;
    the batch epilogue by five) so the PE never head-of-line blocks on the
    scalar/vector engines.
"""
import numpy as np
import ml_dtypes

import concourse.bacc as bacc
import concourse.mybir as mybir
import concourse.tile as tile
from concourse.bass_utils import run_bass_kernel_spmd
from concourse.masks import make_identity

dt = mybir.dt
AF = mybir.ActivationFunctionType
ALU = mybir.AluOpType

L, B, H, HC = 2048, 64, 1024, 1024
NCORES = 8
BPC = B // NCORES        # batches per core
P = 128
LC = L // P              # 16 l-chunks
JH = H // P              # 8 h-chunks
MC = HC // P             # 8 c-chunks
L4 = L // 512            # 4 chunks of 512 along L
LCH = 512                # l-chunk width
NCH = LCH // P           # 4 transpose dmas per l4 tile

_CACHE = {}
BF = ml_dtypes.bfloat16
CHUNK_DEFER = 3   # energy-block slots between a chunk's scores and its softmax work
END_DEFER = 5     # slots between the last chunk and the batch epilogue
TB_BUFS = 2 * L4  # transpose-tile prefetch depth


def _build():
    nc = bacc.Bacc()
    prev = nc.dram_tensor("prev", [L, BPC, H], dt.bfloat16, kind="ExternalInput")
    WoT = nc.dram_tensor("WoT", [P, JH, HC], dt.bfloat16, kind="ExternalInput")
    WhT = nc.dram_tensor("WhT", [P, JH, HC], dt.bfloat16, kind="ExternalInput")
    hT = nc.dram_tensor("hT", [P, JH, BPC], dt.bfloat16, kind="ExternalInput")
    WvT = nc.dram_tensor("WvT", [P, MC], dt.bfloat16, kind="ExternalInput")
    beT = nc.dram_tensor("beT", [P, MC], dt.float32, kind="ExternalInput")
    maskf = nc.dram_tensor("maskf", [BPC, L], dt.float32, kind="ExternalInput")
    out = nc.dram_tensor("out", [BPC, JH, P], dt.float32, kind="ExternalOutput")

    with tile.TileContext(nc) as tc:
        with (
            tc.tile_pool(name="const", bufs=1) as const,
            tc.tile_pool(name="data", bufs=TB_BUFS) as data,
            tc.tile_pool(name="et", bufs=3) as etp,
            tc.tile_pool(name="small", bufs=2) as small,
            tc.tile_pool(name="chnk", bufs=3) as chnk,
            tc.tile_pool(name="pse", bufs=2, space="PSUM") as pse_p,
            tc.tile_pool(name="pss", bufs=2, space="PSUM") as pss_p,
            tc.tile_pool(name="psr", bufs=1, space="PSUM") as psr_p,
            tc.tile_pool(name="psq", bufs=1, space="PSUM") as psq_p,
            tc.tile_pool(name="pso", bufs=1, space="PSUM") as pso_p,
        ):
            # ---- constants; loaded on the ACT HWDGE ring so they don't queue
            # behind the activation transposes on the SP ring
            wo = const.tile([P, JH, HC], dt.bfloat16)
            nc.sync.dma_start(out=wo[:], in_=WoT[:])
            wh = const.tile([P, JH, HC], dt.bfloat16)
            nc.sync.dma_start(out=wh[:], in_=WhT[:])
            ht = const.tile([P, JH, BPC], dt.bfloat16)
            nc.sync.dma_start(out=ht[:], in_=hT[:])
            wv = const.tile([P, MC], dt.bfloat16)
            nc.sync.dma_start(out=wv[:], in_=WvT[:])
            be = const.tile([P, MC], dt.float32)
            nc.sync.dma_start(out=be[:], in_=beT[:])
            ones_bf = const.tile([1, P], dt.bfloat16)
            nc.vector.memset(ones_bf[:], 1.0)
            ones_f = const.tile([1, P], dt.float32)
            nc.vector.memset(ones_f[:], 1.0)
            ident = const.tile([P, P], dt.float32)
            make_identity(nc, ident[:])
            qb = const.tile([P, MC, BPC], dt.float32)

            def make_q(m):
                # q[b, c] = hidden[b] @ W_h.T + b_e, laid out [c-part, m, b]
                def q():
                    psq = psq_p.tile([P, BPC], dt.float32, tag="psq")
                    for u in range(JH):
                        nc.tensor.matmul(
                            psq[:],
                            wh[:, u, m * P:(m + 1) * P],
                            ht[:, u, :],
                            start=(u == 0), stop=(u == JH - 1),
                        )
                    nc.vector.tensor_scalar_add(qb[:, m, :], psq[:],
                                                be[:, m:m + 1])
                return q

            # ---- deferred-emission scheduler over energy-block slots.
            # Global block index g = (b*L4 + l4)*MC + m; sched[g] holds thunks
            # emitted right after energy block g.
            sched = {}
            NBLK = BPC * L4 * MC

            def defer(g, thunk):
                if g >= NBLK:
                    sched.setdefault(NBLK, []).append(thunk)
                else:
                    sched.setdefault(g, []).append(thunk)

            state = {}

            def make_chunk(b, l4, es, m01, tb4, wsum4, s4):
                """Masked exp-weights for one l4 chunk + partial weighted sum."""
                def chunk():
                    lsl = slice(l4 * LCH, (l4 + 1) * LCH)
                    # wtf = es*mask (f32); partial weight-sum for this chunk
                    wtf = chnk.tile([1, LCH], dt.float32, tag="wtf")
                    nc.vector.tensor_mul(wtf[:], es[0:1, lsl], m01[0:1, lsl])
                    nc.vector.reduce_sum(s4[0:1, l4:l4 + 1], wtf[:],
                                         axis=mybir.AxisListType.X)
                    wnb = chnk.tile([1, LCH], dt.bfloat16, tag="wnb")
                    nc.vector.tensor_copy(wnb[:], wtf[:])
                    # broadcast weights to all partitions (K=1 ones matmul)
                    psr = psr_p.tile([P, LCH], dt.float32, tag="psr")
                    nc.tensor.matmul(psr[:], ones_bf[:], wnb[:], start=True, stop=True)
                    wrep = chnk.tile([P, LCH], dt.bfloat16, tag="wrep")
                    nc.vector.tensor_copy(wrep[:], psr[:])
                    # partial weighted sum on DVE:
                    # wsum4[p, j, l4] = sum_l tb4[p, j, l] * wrep[p, l]
                    junk = chnk.tile([P, LCH], dt.float32, tag="ttrjunk")
                    for j in range(JH):
                        nc.vector.tensor_mul(junk[:], tb4[:, j, :], wrep[:])
                        nc.vector.reduce_sum(wsum4[:, j, l4:l4 + 1], junk[:],
                                             axis=mybir.AxisListType.X)
                return chunk

            def make_end(b, wsum4, s4):
                def end():
                    ssum = small.tile([1, 1], dt.float32, tag="ssum")
                    nc.vector.reduce_sum(ssum[:], s4[:], axis=mybir.AxisListType.X)
                    wsum = small.tile([P, JH], dt.float32, tag="wsum")
                    nc.vector.reduce_sum(wsum[:], wsum4[:], axis=mybir.AxisListType.X)
                    rsum = small.tile([1, 1], dt.float32, tag="rsum")
                    nc.vector.reciprocal(rsum[:], ssum[:])
                    # broadcast 1/sum to 128 partitions (K=1 matmul)
                    psb = pso_p.tile([P, JH], dt.float32, tag="psb")
                    nc.tensor.matmul(psb[:, 0:1], ones_f[:], rsum[:],
                                     start=True, stop=True)
                    rsp = small.tile([P, 1], dt.float32, tag="rsp")
                    nc.vector.tensor_copy(rsp[:], psb[:, 0:1])
                    wfin = small.tile([P, JH], dt.float32, tag="wfin")
                    nc.vector.tensor_scalar_mul(wfin[:], wsum[:], rsp[:])
                    # transpose [128, 8] -> [8, 128] and write out
                    pst = pso_p.tile([JH, P], dt.float32, tag="pso")
                    nc.tensor.transpose(pst[:], wfin[:], ident[:])
                    ob = small.tile([JH, P], dt.float32, tag="ob")
                    nc.vector.tensor_copy(ob[:], pst[:])
                    nc.sync.dma_start(out=out[b], in_=ob[:])
                return end

            # ---- main emission loop
            for b in range(BPC):
                # per-l4 transposed tiles: T[p, j, l] = outs[l4*512 + l, 128j + p]
                tb4s = []
                for l4 in range(L4):
                    tb4 = data.tile([P, JH, LCH], dt.bfloat16, tag="tb")
                    for cc in range(NCH):
                        c = l4 * NCH + cc
                        nc.sync.dma_start(
                            out=tb4[:, :, cc * P:(cc + 1) * P],
                            in_=prev[c * P:(c + 1) * P, b, :],
                            transpose=True,
                        )
                    tb4s.append(tb4)
                m01 = small.tile([1, L], dt.float32, tag="m01")
                nc.sync.dma_start(out=m01[:], in_=maskf[b:b + 1, :])
                if b == 0:
                    # q matmuls spread across the first batch's energy slots;
                    # tanh(m) of block m needs qb[:, m] right after block m.
                    for m in range(MC):
                        defer(m, make_q(m))

                es = small.tile([1, L], dt.float32, tag="es")
                wsum4 = small.tile([P, JH, L4], dt.float32, tag="wsum4")
                s4 = small.tile([1, L4], dt.float32, tag="s4")

                for l4 in range(L4):
                    tb4 = tb4s[l4]
                    pss = pss_p.tile([1, LCH], dt.float32, tag="pss")
                    for m in range(MC):
                        g = (b * L4 + l4) * MC + m
                        pse = pse_p.tile([P, LCH], dt.float32, tag="pse")
                        for j in range(JH):
                            nc.tensor.matmul(
                                pse[:],
                                wo[:, j, m * P:(m + 1) * P],
                                tb4[:, j, :],
                                start=(j == 0), stop=(j == JH - 1),
                            )
                        for thunk in sched.pop(g, []):
                            thunk()
                        et = etp.tile([P, LCH], dt.bfloat16, tag="et")
                        nc.scalar.activation(et[:], pse[:], AF.Tanh,
                                             bias=qb[:, m, b:b + 1])

                        def make_s(et=et, pss=pss, m=m, es=es, l4=l4):
                            def s():
                                nc.tensor.matmul(
                                    pss[:], wv[:, m:m + 1], et[:],
                                    start=(m == 0), stop=(m == MC - 1),
                                )
                                if m == MC - 1:
                                    nc.scalar.activation(
                                        es[0:1, l4 * LCH:(l4 + 1) * LCH],
                                        pss[:], AF.Exp)
                            return s
                        defer(g + 1, make_s())
                        if m == MC - 1:
                            defer(g + CHUNK_DEFER, make_chunk(
                                b, l4, es, m01, tb4, wsum4, s4))
                            if l4 == L4 - 1:
                                defer(g + END_DEFER, make_end(b, wsum4, s4))

            for g in sorted(sched):
                for thunk in sched[g]:
                    thunk()

    nc.finalize()
    return nc


def _in_maps(prev_layer_outputs, hidden, mask, W_e, b_e, W_v):
    # host-side layout prep (cheap, O(MB) except the bf16 cast of prev)
    WoT = np.ascontiguousarray(
        W_e[:, H:].T.reshape(JH, P, HC).transpose(1, 0, 2)).astype(BF)
    WhT = np.ascontiguousarray(
        W_e[:, :H].T.reshape(JH, P, HC).transpose(1, 0, 2)).astype(BF)
    hT_full = np.ascontiguousarray(
        hidden.T.reshape(JH, P, B).transpose(1, 0, 2)).astype(BF)
    WvT = np.ascontiguousarray(W_v.reshape(MC, P).T).astype(BF)
    beT = np.ascontiguousarray(b_e.reshape(MC, P).T).astype(np.float32)

    in_maps = []
    for i in range(NCORES):
        bs = slice(i * BPC, (i + 1) * BPC)
        prev_i = np.ascontiguousarray(prev_layer_outputs[:, bs, :]).astype(BF)
        maskf_i = np.ascontiguousarray(mask[bs, :]).astype(np.float32)
        hT_i = np.ascontiguousarray(hT_full[:, :, bs])
        in_maps.append({
            "prev": prev_i, "WoT": WoT, "WhT": WhT, "hT": hT_i,
            "WvT": WvT, "beT": beT, "maskf": maskf_i,
        })
    return in_maps


def kernel(prev_layer_outputs, hidden, mask, W_e, b_e, W_v):
    if "nc" not in _CACHE:
        _CACHE["nc"] = _build()
    nc = _CACHE["nc"]
    in_maps = _in_maps(prev_layer_outputs, hidden, mask, W_e, b_e, W_v)
    res = run_bass_kernel_spmd(nc, in_maps, list(range(NCORES)))
    out = np.concatenate(
        [np.asarray(r["out"]).reshape(1, BPC, H) for r in res.results], axis=1)
    return out.astype(np.float32)


def run_traced(inputs):
    """Profiled run (test harness only)."""
    if "nc" not in _CACHE:
        _CACHE["nc"] = _build()
    nc = _CACHE["nc"]
    in_maps = _in_maps(**inputs)
    return run_bass_kernel_spmd(nc, in_maps, list(range(NCORES)), trace=True)


# revision 22
# speedup vs baseline: 1.1175x; 1.0242x over previous
"""Trainium2 Bass kernel for nn_Attention_72791105732908 (sparse_attention).

Reference computation (L=2048, B=64, H=1024, HC=1024):
    outs   = prev_layer_outputs.transpose(1, 0, 2)              # [B, L, H]
    energy = tanh(concat([hidden_bcast, outs], -1) @ W_e.T + b_e)  # [B, L, HC]
    attn   = energy @ W_v                                        # [B, L]
    attn   = where(mask == 0, -1e10, attn); softmax over L
    out    = einsum('bl,blh->bh', attn, outs)[None]              # [1, B, H]

Strategy:
  - Data-parallel over batch: core i handles batches 8i..8i+7. No collectives.
  - Split the concat matmul: q[b] = hidden[b] @ W_h.T + b_e is computed once
    per batch (tiny); the big matmul is outs @ W_o.T (halves the FLOPs).
  - bf16 on the PE with fp32 PSUM accumulation.
  - outs arrives [L, b, H]; the energy matmul contracts over H, so outs is
    transposed to [H, L] tiles with the DMA xbar (2-byte dtype, DRAM->SBUF,
    mapping T[p, j, l] = outs[l, 128j + p]). Host pre-permutes W to match.
  - Masked softmax without max-subtraction (scores are bounded: |s| <= 32):
    w = exp(s) * mask; the normalization is applied to the reduced output.
  - The weighted sum over L runs on the (otherwise idle) vector engine as
    tensor_tensor_reduce over the transposed tiles, with the masked weights
    broadcast to all partitions by a K=1 ones matmul. This keeps the tensor
    engine free for the energy matmul, which is the roofline.
  - All cross-engine consumers of PE results are deferred on the PE queue
    (scores-MMs by one energy block; softmax/weighted-sum chunks by two;
    the batch epilogue by five) so the PE never head-of-line blocks on the
    scalar/vector engines.
"""
import numpy as np
import ml_dtypes

import concourse.bacc as bacc
import concourse.mybir as mybir
import concourse.tile as tile
from concourse.bass_utils import run_bass_kernel_spmd
from concourse.masks import make_identity

dt = mybir.dt
AF = mybir.ActivationFunctionType
ALU = mybir.AluOpType

L, B, H, HC = 2048, 64, 1024, 1024
NCORES = 8
BPC = B // NCORES        # batches per core
P = 128
LC = L // P              # 16 l-chunks
JH = H // P              # 8 h-chunks
MC = HC // P             # 8 c-chunks
L4 = L // 512            # 4 chunks of 512 along L
LCH = 512                # l-chunk width
NCH = LCH // P           # 4 transpose dmas per l4 tile

_CACHE = {}
BF = ml_dtypes.bfloat16
CHUNK_DEFER = 3   # energy-block slots between a chunk's scores and its softmax work
END_DEFER = 5     # slots between the last chunk and the batch epilogue
TB_BUFS = 2 * L4  # transpose-tile prefetch depth


def _build():
    nc = bacc.Bacc()
    prev = nc.dram_tensor("prev", [L, BPC, H], dt.bfloat16, kind="ExternalInput")
    WoT = nc.dram_tensor("WoT", [P, JH, HC], dt.bfloat16, kind="ExternalInput")
    WhT = nc.dram_tensor("WhT", [P, JH, HC], dt.bfloat16, kind="ExternalInput")
    hT = nc.dram_tensor("hT", [P, JH, BPC], dt.bfloat16, kind="ExternalInput")
    WvT = nc.dram_tensor("WvT", [P, MC], dt.bfloat16, kind="ExternalInput")
    beT = nc.dram_tensor("beT", [P, MC], dt.float32, kind="ExternalInput")
    maskf = nc.dram_tensor("maskf", [BPC, L], dt.float32, kind="ExternalInput")
    out = nc.dram_tensor("out", [BPC, JH, P], dt.float32, kind="ExternalOutput")

    with tile.TileContext(nc) as tc:
        with (
            tc.tile_pool(name="const", bufs=1) as const,
            tc.tile_pool(name="data", bufs=TB_BUFS) as data,
            tc.tile_pool(name="et", bufs=3) as etp,
            tc.tile_pool(name="small", bufs=2) as small,
            tc.tile_pool(name="chnk", bufs=3) as chnk,
            tc.tile_pool(name="pse", bufs=2, space="PSUM") as pse_p,
            tc.tile_pool(name="pss", bufs=2, space="PSUM") as pss_p,
            tc.tile_pool(name="psr", bufs=1, space="PSUM") as psr_p,
            tc.tile_pool(name="psq", bufs=1, space="PSUM") as psq_p,
            tc.tile_pool(name="pso", bufs=1, space="PSUM") as pso_p,
        ):
            # ---- constants; loaded on the ACT HWDGE ring so they don't queue
            # behind the activation transposes on the SP ring
            wo = const.tile([P, JH, HC], dt.bfloat16)
            nc.sync.dma_start(out=wo[:], in_=WoT[:])
            wh = const.tile([P, JH, HC], dt.bfloat16)
            nc.scalar.dma_start(out=wh[:], in_=WhT[:])
            ht = const.tile([P, JH, BPC], dt.bfloat16)
            nc.scalar.dma_start(out=ht[:], in_=hT[:])
            wv = const.tile([P, MC], dt.bfloat16)
            nc.sync.dma_start(out=wv[:], in_=WvT[:])
            be = const.tile([P, MC], dt.float32)
            nc.sync.dma_start(out=be[:], in_=beT[:])
            ones_bf = const.tile([1, P], dt.bfloat16)
            nc.vector.memset(ones_bf[:], 1.0)
            ones_f = const.tile([1, P], dt.float32)
            nc.vector.memset(ones_f[:], 1.0)
            ident = const.tile([P, P], dt.float32)
            make_identity(nc, ident[:])
            qb = const.tile([P, MC, BPC], dt.float32)

            def make_q(m):
                # q[b, c] = hidden[b] @ W_h.T + b_e, laid out [c-part, m, b]
                def q():
                    psq = psq_p.tile([P, BPC], dt.float32, tag="psq")
                    for u in range(JH):
                        nc.tensor.matmul(
                            psq[:],
                            wh[:, u, m * P:(m + 1) * P],
                            ht[:, u, :],
                            start=(u == 0), stop=(u == JH - 1),
                        )
                    nc.vector.tensor_scalar_add(qb[:, m, :], psq[:],
                                                be[:, m:m + 1])
                return q

            # ---- deferred-emission scheduler over energy-block slots.
            # Global block index g = (b*L4 + l4)*MC + m; sched[g] holds thunks
            # emitted right after energy block g.
            sched = {}
            NBLK = BPC * L4 * MC

            def defer(g, thunk):
                if g >= NBLK:
                    sched.setdefault(NBLK, []).append(thunk)
                else:
                    sched.setdefault(g, []).append(thunk)

            state = {}

            def make_chunk(b, l4, es, m01, tb4, wsum4, s4):
                """Masked exp-weights for one l4 chunk + partial weighted sum."""
                def chunk():
                    lsl = slice(l4 * LCH, (l4 + 1) * LCH)
                    # wtf = es*mask (f32); partial weight-sum for this chunk
                    wtf = chnk.tile([1, LCH], dt.float32, tag="wtf")
                    nc.vector.tensor_mul(wtf[:], es[0:1, lsl], m01[0:1, lsl])
                    nc.vector.reduce_sum(s4[0:1, l4:l4 + 1], wtf[:],
                                         axis=mybir.AxisListType.X)
                    wnb = chnk.tile([1, LCH], dt.bfloat16, tag="wnb")
                    nc.vector.tensor_copy(wnb[:], wtf[:])
                    # broadcast weights to all partitions (K=1 ones matmul)
                    psr = psr_p.tile([P, LCH], dt.float32, tag="psr")
                    nc.tensor.matmul(psr[:], ones_bf[:], wnb[:], start=True, stop=True)
                    wrep = chnk.tile([P, LCH], dt.bfloat16, tag="wrep")
                    nc.vector.tensor_copy(wrep[:], psr[:])
                    # partial weighted sum on DVE:
                    # wsum4[p, j, l4] = sum_l tb4[p, j, l] * wrep[p, l]
                    junk = chnk.tile([P, LCH], dt.float32, tag="ttrjunk")
                    for j in range(JH):
                        nc.vector.tensor_mul(junk[:], tb4[:, j, :], wrep[:])
                        nc.vector.reduce_sum(wsum4[:, j, l4:l4 + 1], junk[:],
                                             axis=mybir.AxisListType.X)
                return chunk

            def make_end(b, wsum4, s4):
                def end():
                    ssum = small.tile([1, 1], dt.float32, tag="ssum")
                    nc.vector.reduce_sum(ssum[:], s4[:], axis=mybir.AxisListType.X)
                    wsum = small.tile([P, JH], dt.float32, tag="wsum")
                    nc.vector.reduce_sum(wsum[:], wsum4[:], axis=mybir.AxisListType.X)
                    rsum = small.tile([1, 1], dt.float32, tag="rsum")
                    nc.vector.reciprocal(rsum[:], ssum[:])
                    # broadcast 1/sum to 128 partitions (K=1 matmul)
                    psb = pso_p.tile([P, JH], dt.float32, tag="psb")
                    nc.tensor.matmul(psb[:, 0:1], ones_f[:], rsum[:],
                                     start=True, stop=True)
                    rsp = small.tile([P, 1], dt.float32, tag="rsp")
                    nc.vector.tensor_copy(rsp[:], psb[:, 0:1])
                    wfin = small.tile([P, JH], dt.float32, tag="wfin")
                    nc.vector.tensor_scalar_mul(wfin[:], wsum[:], rsp[:])
                    # transpose [128, 8] -> [8, 128] and write out
                    pst = pso_p.tile([JH, P], dt.float32, tag="pso")
                    nc.tensor.transpose(pst[:], wfin[:], ident[:])
                    ob = small.tile([JH, P], dt.float32, tag="ob")
                    nc.vector.tensor_copy(ob[:], pst[:])
                    nc.sync.dma_start(out=out[b], in_=ob[:])
                return end

            # ---- main emission loop
            for b in range(BPC):
                # per-l4 transposed tiles: T[p, j, l] = outs[l4*512 + l, 128j + p]
                tb4s = []
                for l4 in range(L4):
                    tb4 = data.tile([P, JH, LCH], dt.bfloat16, tag="tb")
                    for cc in range(NCH):
                        c = l4 * NCH + cc
                        nc.sync.dma_start(
                            out=tb4[:, :, cc * P:(cc + 1) * P],
                            in_=prev[c * P:(c + 1) * P, b, :],
                            transpose=True,
                        )
                    tb4s.append(tb4)
                m01 = small.tile([1, L], dt.float32, tag="m01")
                nc.sync.dma_start(out=m01[:], in_=maskf[b:b + 1, :])
                if b == 0:
                    # q matmuls spread across the first batch's energy slots;
                    # tanh(m) of block m needs qb[:, m] right after block m.
                    for m in range(MC):
                        defer(m, make_q(m))

                es = small.tile([1, L], dt.float32, tag="es")
                wsum4 = small.tile([P, JH, L4], dt.float32, tag="wsum4")
                s4 = small.tile([1, L4], dt.float32, tag="s4")

                for l4 in range(L4):
                    tb4 = tb4s[l4]
                    pss = pss_p.tile([1, LCH], dt.float32, tag="pss")
                    for m in range(MC):
                        g = (b * L4 + l4) * MC + m
                        pse = pse_p.tile([P, LCH], dt.float32, tag="pse")
                        for j in range(JH):
                            nc.tensor.matmul(
                                pse[:],
                                wo[:, j, m * P:(m + 1) * P],
                                tb4[:, j, :],
                                start=(j == 0), stop=(j == JH - 1),
                            )
                        for thunk in sched.pop(g, []):
                            thunk()
                        et = etp.tile([P, LCH], dt.bfloat16, tag="et")
                        nc.scalar.activation(et[:], pse[:], AF.Tanh,
                                             bias=qb[:, m, b:b + 1])

                        def make_s(et=et, pss=pss, m=m, es=es, l4=l4):
                            def s():
                                nc.tensor.matmul(
                                    pss[:], wv[:, m:m + 1], et[:],
                                    start=(m == 0), stop=(m == MC - 1),
                                )
                                if m == MC - 1:
                                    nc.scalar.activation(
                                        es[0:1, l4 * LCH:(l4 + 1) * LCH],
                                        pss[:], AF.Exp)
                            return s
                        defer(g + 1, make_s())
                        if m == MC - 1:
                            defer(g + CHUNK_DEFER, make_chunk(
                                b, l4, es, m01, tb4, wsum4, s4))
                            if l4 == L4 - 1:
                                defer(g + END_DEFER, make_end(b, wsum4, s4))

            for g in sorted(sched):
                for thunk in sched[g]:
                    thunk()

    nc.finalize()
    return nc


def _in_maps(prev_layer_outputs, hidden, mask, W_e, b_e, W_v):
    # host-side layout prep (cheap, O(MB) except the bf16 cast of prev)
    WoT = np.ascontiguousarray(
        W_e[:, H:].T.reshape(JH, P, HC).transpose(1, 0, 2)).astype(BF)
    WhT = np.ascontiguousarray(
        W_e[:, :H].T.reshape(JH, P, HC).transpose(1, 0, 2)).astype(BF)
    hT_full = np.ascontiguousarray(
        hidden.T.reshape(JH, P, B).transpose(1, 0, 2)).astype(BF)
    WvT = np.ascontiguousarray(W_v.reshape(MC, P).T).astype(BF)
    beT = np.ascontiguousarray(b_e.reshape(MC, P).T).astype(np.float32)

    def _shard(i):
        bs = slice(i * BPC, (i + 1) * BPC)
        prev_i = prev_layer_outputs[:, bs, :].astype(BF)
        maskf_i = np.ascontiguousarray(mask[bs, :]).astype(np.float32)
        hT_i = np.ascontiguousarray(hT_full[:, :, bs])
        return {
            "prev": prev_i, "WoT": WoT, "WhT": WhT, "hT": hT_i,
            "WvT": WvT, "beT": beT, "maskf": maskf_i,
        }

    from concurrent.futures import ThreadPoolExecutor
    with ThreadPoolExecutor(NCORES) as ex:
        in_maps = list(ex.map(_shard, range(NCORES)))
    return in_maps


def kernel(prev_layer_outputs, hidden, mask, W_e, b_e, W_v):
    if "nc" not in _CACHE:
        _CACHE["nc"] = _build()
    nc = _CACHE["nc"]
    in_maps = _in_maps(prev_layer_outputs, hidden, mask, W_e, b_e, W_v)
    res = run_bass_kernel_spmd(nc, in_maps, list(range(NCORES)))
    out = np.concatenate(
        [np.asarray(r["out"]).reshape(1, BPC, H) for r in res.results], axis=1)
    return out.astype(np.float32)


def run_traced(inputs):
    """Profiled run (test harness only)."""
    if "nc" not in _CACHE:
        _CACHE["nc"] = _build()
    nc = _CACHE["nc"]
    in_maps = _in_maps(**inputs)
    return run_bass_kernel_spmd(nc, in_maps, list(range(NCORES)), trace=True)


# revision 24
# speedup vs baseline: 1.1342x; 1.0149x over previous
"""Trainium2 Bass kernel for nn_Attention_72791105732908 (sparse_attention).

Reference computation (L=2048, B=64, H=1024, HC=1024):
    outs   = prev_layer_outputs.transpose(1, 0, 2)              # [B, L, H]
    energy = tanh(concat([hidden_bcast, outs], -1) @ W_e.T + b_e)  # [B, L, HC]
    attn   = energy @ W_v                                        # [B, L]
    attn   = where(mask == 0, -1e10, attn); softmax over L
    out    = einsum('bl,blh->bh', attn, outs)[None]              # [1, B, H]

Strategy:
  - Data-parallel over batch: core i handles batches 8i..8i+7. No collectives.
  - Split the concat matmul: q[b] = hidden[b] @ W_h.T + b_e is computed once
    per batch (tiny); the big matmul is outs @ W_o.T (halves the FLOPs).
  - bf16 on the PE with fp32 PSUM accumulation.
  - outs arrives [L, b, H]; the energy matmul contracts over H, so outs is
    transposed to [H, L] tiles with the DMA xbar (2-byte dtype, DRAM->SBUF,
    mapping T[p, j, l] = outs[l, 128j + p]). Host pre-permutes W to match.
  - Masked softmax without max-subtraction (scores are bounded: |s| <= 32):
    w = exp(s) * mask; the normalization is applied to the reduced output.
  - The weighted sum over L runs on the (otherwise idle) vector engine as
    tensor_mul + reduce_sum pairs over the transposed tiles, with the masked
    weights broadcast to all partitions by a K=1 ones matmul. This keeps the
    tensor engine free for the energy matmul, which is the roofline.
    (tensor_tensor_reduce would fuse the pair but crashes this runtime.)
  - All cross-engine consumers of PE results are deferred on the PE queue
    (scores-MMs by one energy block; softmax/weighted-sum chunks by three;
    the batch epilogue by five) so the PE never head-of-line blocks on the
    scalar/vector engines.
"""
import numpy as np
import ml_dtypes

import concourse.bacc as bacc
import concourse.mybir as mybir
import concourse.tile as tile
from concourse.bass_utils import run_bass_kernel_spmd
from concourse.masks import make_identity

dt = mybir.dt
AF = mybir.ActivationFunctionType
ALU = mybir.AluOpType

L, B, H, HC = 2048, 64, 1024, 1024
NCORES = 8
BPC = B // NCORES        # batches per core
P = 128
LC = L // P              # 16 l-chunks
JH = H // P              # 8 h-chunks
MC = HC // P             # 8 c-chunks
L4 = L // 512            # 4 chunks of 512 along L
LCH = 512                # l-chunk width
NCH = LCH // P           # 4 transpose dmas per l4 tile

_CACHE = {}
BF = ml_dtypes.bfloat16
CHUNK_DEFER = 3   # energy-block slots between a chunk's scores and its softmax work
END_DEFER = 5     # slots between the last chunk and the batch epilogue
TB_BUFS = 2 * L4  # transpose-tile prefetch depth


def _build():
    nc = bacc.Bacc()
    prev = nc.dram_tensor("prev", [L, BPC, H], dt.bfloat16, kind="ExternalInput")
    WoT = nc.dram_tensor("WoT", [P, JH, HC], dt.bfloat16, kind="ExternalInput")
    WhT = nc.dram_tensor("WhT", [P, JH, HC], dt.bfloat16, kind="ExternalInput")
    hT = nc.dram_tensor("hT", [P, JH, BPC], dt.bfloat16, kind="ExternalInput")
    WvT = nc.dram_tensor("WvT", [P, MC], dt.bfloat16, kind="ExternalInput")
    beT = nc.dram_tensor("beT", [P, MC], dt.float32, kind="ExternalInput")
    maskf = nc.dram_tensor("maskf", [BPC, L], dt.float32, kind="ExternalInput")
    out = nc.dram_tensor("out", [BPC, JH, P], dt.float32, kind="ExternalOutput")

    with tile.TileContext(nc) as tc:
        with (
            tc.tile_pool(name="const", bufs=1) as const,
            tc.tile_pool(name="data", bufs=TB_BUFS) as data,
            tc.tile_pool(name="et", bufs=3) as etp,
            tc.tile_pool(name="small", bufs=2) as small,
            tc.tile_pool(name="chnk", bufs=3) as chnk,
            tc.tile_pool(name="pse", bufs=2, space="PSUM") as pse_p,
            tc.tile_pool(name="pss", bufs=2, space="PSUM") as pss_p,
            tc.tile_pool(name="psr", bufs=1, space="PSUM") as psr_p,
            tc.tile_pool(name="psq", bufs=1, space="PSUM") as psq_p,
            tc.tile_pool(name="pso", bufs=1, space="PSUM") as pso_p,
        ):
            # ---- constants; loaded on the ACT HWDGE ring so they don't queue
            # behind the activation transposes on the SP ring
            wo = const.tile([P, JH, HC], dt.bfloat16)
            nc.sync.dma_start(out=wo[:], in_=WoT[:])
            wh = const.tile([P, JH, HC], dt.bfloat16)
            nc.scalar.dma_start(out=wh[:], in_=WhT[:])
            ht = const.tile([P, JH, BPC], dt.bfloat16)
            nc.scalar.dma_start(out=ht[:], in_=hT[:])
            wv = const.tile([P, MC], dt.bfloat16)
            nc.sync.dma_start(out=wv[:], in_=WvT[:])
            be = const.tile([P, MC], dt.float32)
            nc.sync.dma_start(out=be[:], in_=beT[:])
            ones_bf = const.tile([1, P], dt.bfloat16)
            nc.vector.memset(ones_bf[:], 1.0)
            ones_f = const.tile([1, P], dt.float32)
            nc.vector.memset(ones_f[:], 1.0)
            ident = const.tile([P, P], dt.float32)
            make_identity(nc, ident[:])
            qb = const.tile([P, MC, BPC], dt.float32)

            def make_q(m):
                # q[b, c] = hidden[b] @ W_h.T + b_e, laid out [c-part, m, b]
                def q():
                    psq = psq_p.tile([P, BPC], dt.float32, tag="psq")
                    for u in range(JH):
                        nc.tensor.matmul(
                            psq[:],
                            wh[:, u, m * P:(m + 1) * P],
                            ht[:, u, :],
                            start=(u == 0), stop=(u == JH - 1),
                        )
                    nc.vector.tensor_scalar_add(qb[:, m, :], psq[:],
                                                be[:, m:m + 1])
                return q

            # ---- deferred-emission scheduler over energy-block slots.
            # Global block index g = (b*L4 + l4)*MC + m; sched[g] holds thunks
            # emitted right after energy block g.
            sched = {}
            NBLK = BPC * L4 * MC

            def defer(g, thunk):
                if g >= NBLK:
                    sched.setdefault(NBLK, []).append(thunk)
                else:
                    sched.setdefault(g, []).append(thunk)

            state = {}

            def make_chunk(b, l4, es, m01, tb4, wsum4, s4):
                """Masked exp-weights for one l4 chunk + partial weighted sum."""
                def chunk():
                    lsl = slice(l4 * LCH, (l4 + 1) * LCH)
                    # wtf = es*mask (f32); partial weight-sum for this chunk
                    wtf = chnk.tile([1, LCH], dt.float32, tag="wtf")
                    nc.vector.tensor_mul(wtf[:], es[0:1, lsl], m01[0:1, lsl])
                    nc.vector.reduce_sum(s4[0:1, l4:l4 + 1], wtf[:],
                                         axis=mybir.AxisListType.X)
                    wnb = chnk.tile([1, LCH], dt.bfloat16, tag="wnb")
                    nc.vector.tensor_copy(wnb[:], wtf[:])
                    # broadcast weights to all partitions (K=1 ones matmul)
                    psr = psr_p.tile([P, LCH], dt.float32, tag="psr")
                    nc.tensor.matmul(psr[:], ones_bf[:], wnb[:], start=True, stop=True)
                    wrep = chnk.tile([P, LCH], dt.bfloat16, tag="wrep")
                    nc.vector.tensor_copy(wrep[:], psr[:])
                    # partial weighted sum on DVE:
                    # wsum4[p, j, l4] = sum_l tb4[p, j, l] * wrep[p, l]
                    junk = chnk.tile([P, LCH], dt.float32, tag="ttrjunk")
                    for j in range(JH):
                        nc.vector.tensor_mul(junk[:], tb4[:, j, :], wrep[:])
                        nc.vector.reduce_sum(wsum4[:, j, l4:l4 + 1], junk[:],
                                             axis=mybir.AxisListType.X)
                return chunk

            def make_end(b, wsum4, s4):
                def end():
                    ssum = small.tile([1, 1], dt.float32, tag="ssum")
                    nc.vector.reduce_sum(ssum[:], s4[:], axis=mybir.AxisListType.X)
                    wsum = small.tile([P, JH], dt.float32, tag="wsum")
                    nc.vector.reduce_sum(wsum[:], wsum4[:], axis=mybir.AxisListType.X)
                    rsum = small.tile([1, 1], dt.float32, tag="rsum")
                    nc.vector.reciprocal(rsum[:], ssum[:])
                    # broadcast 1/sum to 128 partitions (K=1 matmul)
                    psb = pso_p.tile([P, JH], dt.float32, tag="psb")
                    nc.tensor.matmul(psb[:, 0:1], ones_f[:], rsum[:],
                                     start=True, stop=True)
                    rsp = small.tile([P, 1], dt.float32, tag="rsp")
                    nc.vector.tensor_copy(rsp[:], psb[:, 0:1])
                    wfin = small.tile([P, JH], dt.float32, tag="wfin")
                    nc.vector.tensor_scalar_mul(wfin[:], wsum[:], rsp[:])
                    # transpose [128, 8] -> [8, 128] and write out
                    pst = pso_p.tile([JH, P], dt.float32, tag="pso")
                    nc.tensor.transpose(pst[:], wfin[:], ident[:])
                    ob = small.tile([JH, P], dt.float32, tag="ob")
                    nc.vector.tensor_copy(ob[:], pst[:])
                    nc.sync.dma_start(out=out[b], in_=ob[:])
                return end

            # ---- main emission loop
            for b in range(BPC):
                # per-l4 transposed tiles: T[p, j, l] = outs[l4*512 + l, 128j + p]
                tb4s = []
                for l4 in range(L4):
                    tb4 = data.tile([P, JH, LCH], dt.bfloat16, tag="tb")
                    for cc in range(NCH):
                        c = l4 * NCH + cc
                        nc.sync.dma_start(
                            out=tb4[:, :, cc * P:(cc + 1) * P],
                            in_=prev[c * P:(c + 1) * P, b, :],
                            transpose=True,
                        )
                    tb4s.append(tb4)
                m01 = small.tile([1, L], dt.float32, tag="m01")
                nc.sync.dma_start(out=m01[:], in_=maskf[b:b + 1, :])
                if b == 0:
                    # q matmuls spread across the first batch's energy slots;
                    # tanh(m) of block m needs qb[:, m] right after block m.
                    for m in range(MC):
                        defer(m, make_q(m))

                es = small.tile([1, L], dt.float32, tag="es")
                wsum4 = small.tile([P, JH, L4], dt.float32, tag="wsum4")
                s4 = small.tile([1, L4], dt.float32, tag="s4")

                for l4 in range(L4):
                    tb4 = tb4s[l4]
                    pss = pss_p.tile([1, LCH], dt.float32, tag="pss")
                    for m in range(MC):
                        g = (b * L4 + l4) * MC + m
                        pse = pse_p.tile([P, LCH], dt.float32, tag="pse")
                        for j in range(JH):
                            nc.tensor.matmul(
                                pse[:],
                                wo[:, j, m * P:(m + 1) * P],
                                tb4[:, j, :],
                                start=(j == 0), stop=(j == JH - 1),
                            )
                        for thunk in sched.pop(g, []):
                            thunk()
                        et = etp.tile([P, LCH], dt.bfloat16, tag="et")
                        nc.scalar.activation(et[:], pse[:], AF.Tanh,
                                             bias=qb[:, m, b:b + 1])

                        def make_s(et=et, pss=pss, m=m, es=es, l4=l4):
                            def s():
                                nc.tensor.matmul(
                                    pss[:], wv[:, m:m + 1], et[:],
                                    start=(m == 0), stop=(m == MC - 1),
                                )
                                if m == MC - 1:
                                    nc.scalar.activation(
                                        es[0:1, l4 * LCH:(l4 + 1) * LCH],
                                        pss[:], AF.Exp)
                            return s
                        defer(g + 1, make_s())
                        if m == MC - 1:
                            defer(g + CHUNK_DEFER, make_chunk(
                                b, l4, es, m01, tb4, wsum4, s4))
                            if l4 == L4 - 1:
                                defer(g + END_DEFER, make_end(b, wsum4, s4))

            for g in sorted(sched):
                for thunk in sched[g]:
                    thunk()

    nc.finalize()
    return nc


def _in_maps(prev_layer_outputs, hidden, mask, W_e, b_e, W_v):
    # host-side layout prep (cheap, O(MB) except the bf16 cast of prev)
    WoT = np.ascontiguousarray(
        W_e[:, H:].T.reshape(JH, P, HC).transpose(1, 0, 2)).astype(BF)
    WhT = np.ascontiguousarray(
        W_e[:, :H].T.reshape(JH, P, HC).transpose(1, 0, 2)).astype(BF)
    hT_full = np.ascontiguousarray(
        hidden.T.reshape(JH, P, B).transpose(1, 0, 2)).astype(BF)
    WvT = np.ascontiguousarray(W_v.reshape(MC, P).T).astype(BF)
    beT = np.ascontiguousarray(b_e.reshape(MC, P).T).astype(np.float32)

    def _shard(i):
        bs = slice(i * BPC, (i + 1) * BPC)
        prev_i = prev_layer_outputs[:, bs, :].astype(BF)
        maskf_i = np.ascontiguousarray(mask[bs, :]).astype(np.float32)
        hT_i = np.ascontiguousarray(hT_full[:, :, bs])
        return {
            "prev": prev_i, "WoT": WoT, "WhT": WhT, "hT": hT_i,
            "WvT": WvT, "beT": beT, "maskf": maskf_i,
        }

    from concurrent.futures import ThreadPoolExecutor
    with ThreadPoolExecutor(NCORES) as ex:
        in_maps = list(ex.map(_shard, range(NCORES)))
    return in_maps


def kernel(prev_layer_outputs, hidden, mask, W_e, b_e, W_v):
    prev_layer_outputs = np.asarray(prev_layer_outputs)
    hidden = np.asarray(hidden)
    mask = np.asarray(mask)
    W_e = np.asarray(W_e)
    b_e = np.asarray(b_e)
    W_v = np.asarray(W_v)
    if "nc" not in _CACHE:
        _CACHE["nc"] = _build()
    nc = _CACHE["nc"]
    in_maps = _in_maps(prev_layer_outputs, hidden, mask, W_e, b_e, W_v)
    res = run_bass_kernel_spmd(nc, in_maps, list(range(NCORES)))
    out = np.concatenate(
        [np.asarray(r["out"]).reshape(1, BPC, H) for r in res.results], axis=1)
    return out.astype(np.float32)


def run_traced(inputs):
    """Profiled run (test harness only)."""
    if "nc" not in _CACHE:
        _CACHE["nc"] = _build()
    nc = _CACHE["nc"]
    in_maps = _in_maps(**inputs)
    return run_bass_kernel_spmd(nc, in_maps, list(range(NCORES)), trace=True)


# revision 25
# speedup vs baseline: 1.3355x; 1.1776x over previous
"""Trainium2 Bass kernel for nn_Attention_72791105732908 (sparse_attention).

Reference computation (L=2048, B=64, H=1024, HC=1024):
    outs   = prev_layer_outputs.transpose(1, 0, 2)              # [B, L, H]
    energy = tanh(concat([hidden_bcast, outs], -1) @ W_e.T + b_e)  # [B, L, HC]
    attn   = energy @ W_v                                        # [B, L]
    attn   = where(mask == 0, -1e10, attn); softmax over L
    out    = einsum('bl,blh->bh', attn, outs)[None]              # [1, B, H]

Strategy:
  - Data-parallel over batch: core i handles batches 8i..8i+7. No collectives.
  - Split the concat matmul: q[b] = hidden[b] @ W_h.T + b_e is computed once
    per batch (tiny); the big matmul is outs @ W_o.T (halves the FLOPs).
  - bf16 on the PE with fp32 PSUM accumulation.
  - outs arrives [L, b, H]; the energy matmul contracts over H, so outs is
    transposed to [H, L] tiles with the DMA xbar (2-byte dtype, DRAM->SBUF,
    mapping T[p, j, l] = outs[l, 128j + p]). Host pre-permutes W to match.
  - Masked softmax without max-subtraction (scores are bounded: |s| <= 32):
    w = exp(s) * mask; the normalization is applied to the reduced output.
  - The weighted sum over L runs on the (otherwise idle) vector engine as
    tensor_mul + reduce_sum pairs over the transposed tiles, with the masked
    weights broadcast to all partitions by a K=1 ones matmul. This keeps the
    tensor engine free for the energy matmul, which is the roofline.
    (tensor_tensor_reduce would fuse the pair but crashes this runtime.)
  - All cross-engine consumers of PE results are deferred on the PE queue
    (scores-MMs by one energy block; softmax/weighted-sum chunks by three;
    the batch epilogue by five) so the PE never head-of-line blocks on the
    scalar/vector engines.
"""
import numpy as np
import ml_dtypes

import concourse.bacc as bacc
import concourse.mybir as mybir
import concourse.tile as tile
from concourse.bass_utils import run_bass_kernel_spmd
from concourse.masks import make_identity

dt = mybir.dt
AF = mybir.ActivationFunctionType
ALU = mybir.AluOpType

L, B, H, HC = 2048, 64, 1024, 1024
NCORES = 8
BPC = B // NCORES        # batches per core
P = 128
LC = L // P              # 16 l-chunks
JH = H // P              # 8 h-chunks
MC = HC // P             # 8 c-chunks
L4 = L // 512            # 4 chunks of 512 along L
LCH = 512                # l-chunk width
NCH = LCH // P           # 4 transpose dmas per l4 tile

_CACHE = {}
BF = ml_dtypes.bfloat16
CHUNK_DEFER = 3   # energy-block slots between a chunk's scores and its softmax work
END_DEFER = 5     # slots between the last chunk and the batch epilogue
TB_BUFS = 2 * L4  # transpose-tile prefetch depth
PSE_BUFS = 2      # energy psum double/triple buffering


def _build():
    nc = bacc.Bacc()
    prev = nc.dram_tensor("prev", [L, BPC, H], dt.bfloat16, kind="ExternalInput")
    WoT = nc.dram_tensor("WoT", [P, JH, HC], dt.bfloat16, kind="ExternalInput")
    WhT = nc.dram_tensor("WhT", [P, JH, HC], dt.bfloat16, kind="ExternalInput")
    hT = nc.dram_tensor("hT", [P, JH, BPC], dt.bfloat16, kind="ExternalInput")
    WvT = nc.dram_tensor("WvT", [P, MC], dt.bfloat16, kind="ExternalInput")
    beT = nc.dram_tensor("beT", [P, MC], dt.float32, kind="ExternalInput")
    maskf = nc.dram_tensor("maskf", [BPC, L], dt.float32, kind="ExternalInput")
    out = nc.dram_tensor("out", [BPC, JH, P], dt.float32, kind="ExternalOutput")

    with tile.TileContext(nc) as tc:
        with (
            tc.tile_pool(name="const", bufs=1) as const,
            tc.tile_pool(name="data", bufs=TB_BUFS) as data,
            tc.tile_pool(name="et", bufs=3) as etp,
            tc.tile_pool(name="small", bufs=2) as small,
            tc.tile_pool(name="chnk", bufs=3) as chnk,
            tc.tile_pool(name="pse", bufs=PSE_BUFS, space="PSUM") as pse_p,
            tc.tile_pool(name="pss", bufs=2, space="PSUM") as pss_p,
            tc.tile_pool(name="psr", bufs=1, space="PSUM") as psr_p,
            tc.tile_pool(name="psq", bufs=1, space="PSUM") as psq_p,
            tc.tile_pool(name="pso", bufs=1, space="PSUM") as pso_p,
        ):
            # ---- constants; loaded on the ACT HWDGE ring so they don't queue
            # behind the activation transposes on the SP ring
            wo = const.tile([P, JH, HC], dt.bfloat16)
            nc.sync.dma_start(out=wo[:], in_=WoT[:])
            wh = const.tile([P, JH, HC], dt.bfloat16)
            nc.scalar.dma_start(out=wh[:], in_=WhT[:])
            ht = const.tile([P, JH, BPC], dt.bfloat16)
            nc.scalar.dma_start(out=ht[:], in_=hT[:])
            wv = const.tile([P, MC], dt.bfloat16)
            nc.sync.dma_start(out=wv[:], in_=WvT[:])
            be = const.tile([P, MC], dt.float32)
            nc.sync.dma_start(out=be[:], in_=beT[:])
            ones_bf = const.tile([1, P], dt.bfloat16)
            nc.vector.memset(ones_bf[:], 1.0)
            ones_f = const.tile([1, P], dt.float32)
            nc.vector.memset(ones_f[:], 1.0)
            ident = const.tile([P, P], dt.float32)
            make_identity(nc, ident[:])
            qb = const.tile([P, MC, BPC], dt.float32)

            def make_q(m):
                # q[b, c] = hidden[b] @ W_h.T + b_e, laid out [c-part, m, b]
                def q():
                    psq = psq_p.tile([P, BPC], dt.float32, tag="psq")
                    for u in range(JH):
                        nc.tensor.matmul(
                            psq[:],
                            wh[:, u, m * P:(m + 1) * P],
                            ht[:, u, :],
                            start=(u == 0), stop=(u == JH - 1),
                        )
                    nc.vector.tensor_scalar_add(qb[:, m, :], psq[:],
                                                be[:, m:m + 1])
                return q

            # ---- deferred-emission scheduler over energy-block slots.
            # Global block index g = (b*L4 + l4)*MC + m; sched[g] holds thunks
            # emitted right after energy block g.
            sched = {}
            NBLK = BPC * L4 * MC

            def defer(g, thunk):
                if g >= NBLK:
                    sched.setdefault(NBLK, []).append(thunk)
                else:
                    sched.setdefault(g, []).append(thunk)

            state = {}

            def make_chunk(b, l4, es, m01, tb4, wsum4, s4):
                """Masked exp-weights for one l4 chunk + partial weighted sum."""
                def chunk():
                    lsl = slice(l4 * LCH, (l4 + 1) * LCH)
                    # wtf = es*mask (f32); partial weight-sum for this chunk
                    wtf = chnk.tile([1, LCH], dt.float32, tag="wtf")
                    nc.vector.tensor_mul(wtf[:], es[0:1, lsl], m01[0:1, lsl])
                    nc.vector.reduce_sum(s4[0:1, l4:l4 + 1], wtf[:],
                                         axis=mybir.AxisListType.X)
                    wnb = chnk.tile([1, LCH], dt.bfloat16, tag="wnb")
                    nc.vector.tensor_copy(wnb[:], wtf[:])
                    # broadcast weights to all partitions (K=1 ones matmul)
                    psr = psr_p.tile([P, LCH], dt.float32, tag="psr")
                    nc.tensor.matmul(psr[:], ones_bf[:], wnb[:], start=True, stop=True)
                    wrep = chnk.tile([P, LCH], dt.bfloat16, tag="wrep")
                    nc.vector.tensor_copy(wrep[:], psr[:])
                    # partial weighted sum on DVE:
                    # wsum4[p, j, l4] = sum_l tb4[p, j, l] * wrep[p, l]
                    junk = chnk.tile([P, LCH], dt.float32, tag="ttrjunk")
                    for j in range(JH):
                        nc.vector.tensor_mul(junk[:], tb4[:, j, :], wrep[:])
                        nc.vector.reduce_sum(wsum4[:, j, l4:l4 + 1], junk[:],
                                             axis=mybir.AxisListType.X)
                return chunk

            def make_end(b, wsum4, s4):
                def end():
                    ssum = small.tile([1, 1], dt.float32, tag="ssum")
                    nc.vector.reduce_sum(ssum[:], s4[:], axis=mybir.AxisListType.X)
                    wsum = small.tile([P, JH], dt.float32, tag="wsum")
                    nc.vector.reduce_sum(wsum[:], wsum4[:], axis=mybir.AxisListType.X)
                    rsum = small.tile([1, 1], dt.float32, tag="rsum")
                    nc.vector.reciprocal(rsum[:], ssum[:])
                    # broadcast 1/sum to 128 partitions (K=1 matmul)
                    psb = pso_p.tile([P, JH], dt.float32, tag="pso")
                    nc.tensor.matmul(psb[:, 0:1], ones_f[:], rsum[:],
                                     start=True, stop=True)
                    rsp = small.tile([P, 1], dt.float32, tag="rsp")
                    nc.vector.tensor_copy(rsp[:], psb[:, 0:1])
                    wfin = small.tile([P, JH], dt.float32, tag="wfin")
                    nc.vector.tensor_scalar_mul(wfin[:], wsum[:], rsp[:])
                    # transpose [128, 8] -> [8, 128] and write out
                    pst = pso_p.tile([JH, P], dt.float32, tag="pso")
                    nc.tensor.transpose(pst[:], wfin[:], ident[:])
                    ob = small.tile([JH, P], dt.float32, tag="ob")
                    nc.vector.tensor_copy(ob[:], pst[:])
                    nc.sync.dma_start(out=out[b], in_=ob[:])
                return end

            # ---- main emission loop
            for b in range(BPC):
                # per-l4 transposed tiles: T[p, j, l] = outs[l4*512 + l, 128j + p]
                tb4s = []
                for l4 in range(L4):
                    tb4 = data.tile([P, JH, LCH], dt.bfloat16, tag="tb")
                    for cc in range(NCH):
                        c = l4 * NCH + cc
                        nc.sync.dma_start(
                            out=tb4[:, :, cc * P:(cc + 1) * P],
                            in_=prev[c * P:(c + 1) * P, b, :],
                            transpose=True,
                        )
                    tb4s.append(tb4)
                m01 = small.tile([1, L], dt.float32, tag="m01")
                nc.sync.dma_start(out=m01[:], in_=maskf[b:b + 1, :])
                if b == 0:
                    # q matmuls spread across the first batch's energy slots;
                    # tanh(m) of block m needs qb[:, m] right after block m.
                    for m in range(MC):
                        defer(m, make_q(m))

                es = small.tile([1, L], dt.float32, tag="es")
                wsum4 = small.tile([P, JH, L4], dt.float32, tag="wsum4")
                s4 = small.tile([1, L4], dt.float32, tag="s4")

                for l4 in range(L4):
                    tb4 = tb4s[l4]
                    pss = pss_p.tile([1, LCH], dt.float32, tag="pss")
                    for m in range(MC):
                        g = (b * L4 + l4) * MC + m
                        pse = pse_p.tile([P, LCH], dt.float32, tag="pse")
                        for j in range(JH):
                            nc.tensor.matmul(
                                pse[:],
                                wo[:, j, m * P:(m + 1) * P],
                                tb4[:, j, :],
                                start=(j == 0), stop=(j == JH - 1),
                            )
                        for thunk in sched.pop(g, []):
                            thunk()
                        et = etp.tile([P, LCH], dt.bfloat16, tag="et")
                        nc.scalar.activation(et[:], pse[:], AF.Tanh,
                                             bias=qb[:, m, b:b + 1])

                        def make_s(et=et, pss=pss, m=m, es=es, l4=l4):
                            def s():
                                nc.tensor.matmul(
                                    pss[:], wv[:, m:m + 1], et[:],
                                    start=(m == 0), stop=(m == MC - 1),
                                )
                                if m == MC - 1:
                                    nc.scalar.activation(
                                        es[0:1, l4 * LCH:(l4 + 1) * LCH],
                                        pss[:], AF.Exp)
                            return s
                        defer(g + 1, make_s())
                        if m == MC - 1:
                            defer(g + CHUNK_DEFER, make_chunk(
                                b, l4, es, m01, tb4, wsum4, s4))
                            if l4 == L4 - 1:
                                defer(g + END_DEFER, make_end(b, wsum4, s4))

            for g in sorted(sched):
                for thunk in sched[g]:
                    thunk()

    nc.finalize()
    return nc


def _in_maps(prev_layer_outputs, hidden, mask, W_e, b_e, W_v):
    # host-side layout prep (cheap, O(MB) except the bf16 cast of prev)
    WoT = np.ascontiguousarray(
        W_e[:, H:].T.reshape(JH, P, HC).transpose(1, 0, 2)).astype(BF)
    WhT = np.ascontiguousarray(
        W_e[:, :H].T.reshape(JH, P, HC).transpose(1, 0, 2)).astype(BF)
    hT_full = np.ascontiguousarray(
        hidden.T.reshape(JH, P, B).transpose(1, 0, 2)).astype(BF)
    WvT = np.ascontiguousarray(W_v.reshape(MC, P).T).astype(BF)
    beT = np.ascontiguousarray(b_e.reshape(MC, P).T).astype(np.float32)

    def _shard(i):
        bs = slice(i * BPC, (i + 1) * BPC)
        prev_i = prev_layer_outputs[:, bs, :].astype(BF)
        maskf_i = np.ascontiguousarray(mask[bs, :]).astype(np.float32)
        hT_i = np.ascontiguousarray(hT_full[:, :, bs])
        return {
            "prev": prev_i, "WoT": WoT, "WhT": WhT, "hT": hT_i,
            "WvT": WvT, "beT": beT, "maskf": maskf_i,
        }

    from concurrent.futures import ThreadPoolExecutor
    with ThreadPoolExecutor(NCORES) as ex:
        in_maps = list(ex.map(_shard, range(NCORES)))
    return in_maps


def kernel(prev_layer_outputs, hidden, mask, W_e, b_e, W_v):
    prev_layer_outputs = np.asarray(prev_layer_outputs)
    hidden = np.asarray(hidden)
    mask = np.asarray(mask)
    W_e = np.asarray(W_e)
    b_e = np.asarray(b_e)
    W_v = np.asarray(W_v)
    if "nc" not in _CACHE:
        _CACHE["nc"] = _build()
    nc = _CACHE["nc"]
    in_maps = _in_maps(prev_layer_outputs, hidden, mask, W_e, b_e, W_v)
    res = run_bass_kernel_spmd(nc, in_maps, list(range(NCORES)))
    out = np.concatenate(
        [np.asarray(r["out"]).reshape(1, BPC, H) for r in res.results], axis=1)
    return out.astype(np.float32)


def run_traced(inputs):
    """Profiled run (test harness only)."""
    if "nc" not in _CACHE:
        _CACHE["nc"] = _build()
    nc = _CACHE["nc"]
    in_maps = _in_maps(**inputs)
    return run_bass_kernel_spmd(nc, in_maps, list(range(NCORES)), trace=True)
